# revision 4
# baseline (speedup 1.0000x reference)
"""3-layer GCN (DGL GraphConv norm='both') on 8 TRN2 NeuronCores — v2.

Strategy (edge-cut, dst-owner sharding), v2 changes over v1:
  - fp16 node-feature tables: gather rows are 256B (was 512B), halving the
    dominant HBM gather traffic; segment-sum and W matmuls run fp16 (2x PE).
  - Variable-length gather calls per (slice, chunk) with trailing -1 idx
    padding (ignored by the gather HW), instead of fixed 1024-idx pair
    calls: removes ~19% padded gather traffic. Call lengths are shared
    across cores (SPMD program), per-core shortfall padded with -1.
  - One-hot S_T build: ONE wide DVE tensor_tensor(is_equal) per slice
    ([128, nt*128] vs nt separate tensor_scalar ops), with the rel scalars
    read through a stride-0 broadcast AP.
  - PSUM->SBUF agg copy on the scalar engine (activation Copy, fp16 out).
  - AllGather split into 4 segments, each issued as soon as its 30 slices
    are computed, overlapping collective with compute. The node permutation
    is quarter-major (global slice = q*240 + core*30 + j) so AG segment q
    reassembles exactly gather chunk q. AG outputs use Shared scratchpad.
"""

import os as _os

import numpy as np

import concourse.bass as bass
import concourse.mybir as mybir
import concourse.tile as tile
from concourse import bacc, library_config
from concourse.bass_utils import run_bass_kernel_spmd

P = 128
NCORES = 8
N_NODES = 100000
SLICES_TOTAL = 960
SLICES_CORE = SLICES_TOTAL // NCORES  # 120
N_PAD = SLICES_TOTAL * P  # 122880
PER_CORE = SLICES_CORE * P  # 15360
NCHUNK = 4
CHUNK_ROWS = N_PAD // NCHUNK  # 30720 < 32768 (int16 idx limit)
SLOT_CAP = 512  # max edges per (slice, chunk) bucket
NSPLIT = 4  # AllGather segments per layer
SLICES_SEG = SLICES_CORE // NSPLIT  # 30
SEG_ROWS = SLICES_SEG * P  # 3840 rows of ag_in per segment
F_IN = 128
F_HID = 128
F_OUT = 64
MAXT = SLOT_CAP // P  # 4 gather blocks per (slice, chunk)

_DBG_LAYERS = int(_os.environ.get("GNN_LAYERS", "3"))
_DBG_NO_AG = int(_os.environ.get("GNN_NO_AG", "0"))
_DBG_REPS = int(_os.environ.get("GNN_REPS", "1"))
_DBG_UNROLL = int(_os.environ.get("GNN_REPS_UNROLL", "1"))
_SKIP_GATHER = int(_os.environ.get("GNN_SKIP_GATHER", "0"))
_SKIP_COMPUTE = int(_os.environ.get("GNN_SKIP_COMPUTE", "0"))
_NQUEUES = int(_os.environ.get("GNN_NQUEUES", "4"))
_AG_IN_LOOP = int(_os.environ.get("GNN_AG_IN_LOOP", "0"))
_SKIP_ONEHOT = int(_os.environ.get("GNN_SKIP_ONEHOT", "0"))
# 0 = all one-hots on DVE; N>0 = every Nth slice's one-hot on GpSimd (Pool)
_OH_POOL_EVERY = int(_os.environ.get("GNN_OH_POOL", "0"))

FP16 = mybir.dt.float16


def _preprocess(src, dst):
    """Permutation + per-bucket gather indices / dst slots.

    Returns per-core call metadata shared across cores (ni per call) and
    per-core idx/rel payloads.
    """
    n = N_NODES
    deg_out = np.bincount(src, minlength=n).astype(np.float32)
    deg_in = np.bincount(dst, minlength=n).astype(np.float32)
    ns = 1.0 / np.sqrt(np.maximum(deg_out, 1.0))
    nd = 1.0 / np.sqrt(np.maximum(deg_in, 1.0))

    # Degree-balanced global slice assignment: snake round-robin over the
    # 960 slices in descending in-degree order.
    order = np.argsort(-deg_in, kind="stable")
    gs_of = np.empty(n, dtype=np.int64)
    slot_of = np.empty(n, dtype=np.int64)
    for k in range(0, n, SLICES_TOTAL):
        stratum = order[k : k + SLICES_TOTAL]
        slot = k // SLICES_TOTAL
        m = len(stratum)
        if (slot % 2) == 0:
            slices = np.arange(m)
        else:
            slices = SLICES_TOTAL - 1 - np.arange(m)
        gs_of[stratum] = slices
        slot_of[stratum] = slot

    # Quarter-major layout: global slice gs = q*240 + core*30 + j, so that
    # AllGather over segment q (each core's slices q*30..q*30+29, in core
    # order) lands exactly on table chunk q.
    core_of = (gs_of % (NCORES * SLICES_SEG)) // SLICES_SEG
    sloc_of = (gs_of // (NCORES * SLICES_SEG)) * SLICES_SEG + (gs_of % SLICES_SEG)
    trow = gs_of * P + slot_of  # row in the replicated feature table
    catrow = core_of * PER_CORE + sloc_of * P + slot_of  # row in concat(out)

    s_perm = trow[src]
    chunk = s_perm // CHUNK_ROWS
    idxval = (s_perm % CHUNK_ROWS).astype(np.int16)
    slotd = slot_of[dst].astype(np.float32)
    # bucket key: (owner core, local slice, chunk)
    key = (core_of[dst] * SLICES_CORE + sloc_of[dst]) * NCHUNK + chunk

    nbuckets = SLICES_TOTAL * NCHUNK
    counts = np.bincount(key, minlength=nbuckets)
    if counts.max() > SLOT_CAP:
        raise RuntimeError(f"bucket overflow: {counts.max()} > {SLOT_CAP}")
    eorder = np.argsort(key, kind="stable")
    offs = np.zeros(nbuckets + 1, dtype=np.int64)
    np.cumsum(counts, out=offs[1:])
    pos = np.arange(len(src)) - offs[key[eorder]]

    # idx pad = 0 (a valid row; its contribution is killed by rel = -1,
    # whose one-hot column is all zeros). All ni slots must be valid
    # indices because num_idxs_reg is baked into the SPMD program.
    idx_pad = np.zeros((nbuckets, SLOT_CAP), dtype=np.int16)
    rel_pad = np.full((nbuckets, SLOT_CAP), -1.0, dtype=np.float32)
    idx_pad[key[eorder], pos] = idxval[eorder]
    rel_pad[key[eorder], pos] = slotd[eorder]

    counts = counts.reshape(NCORES, SLICES_CORE, NCHUNK)
    # shared call geometry: per (slice, chunk), ni = roundup(max_core cnt, 16)
    maxcnt = counts.max(axis=0)  # [SLICES_CORE, NCHUNK]
    ni_call = ((maxcnt + 15) // 16) * 16  # 0 if all cores empty
    nblk_call = (ni_call + 127) // 128

    return dict(
        ns=ns,
        nd=nd,
        trow=trow,
        catrow=catrow,
        core_of=core_of,
        sloc_of=sloc_of,
        slot_of=slot_of,
        idx_pad=idx_pad,
        rel_pad=rel_pad,
        ni_call=ni_call,
        nblk_call=nblk_call,
    )


def _wrap16(flat):
    """[NI] int16 -> [128, NI//16]: element j at [j%16, j//16], replicated x8."""
    w = flat.reshape(-1, 16).T
    return np.tile(w, (8, 1))


def _geometry(pre):
    """Shared (cross-core) call/column geometry derived from ni/nblk."""
    ni = pre["ni_call"]
    nblk = pre["nblk_call"]
    icols = int(ni.sum() // 16)
    relcols = int(nblk.sum())
    return icols, relcols


def _core_arrays(core, pre):
    """Per-core idx/rel/norm payloads matching the shared device loop order."""
    ni_call = pre["ni_call"]
    nblk_call = pre["nblk_call"]
    idx_pad = pre["idx_pad"]
    rel_pad = pre["rel_pad"]
    icols, relcols = _geometry(pre)

    idx_all = np.zeros((P, icols), dtype=np.int16)
    rel_cols = np.full((P, relcols), -1.0, dtype=np.float16)
    icol = 0
    col = 0
    for s in range(SLICES_CORE):
        for c in range(NCHUNK):
            b = (core * SLICES_CORE + s) * NCHUNK + c
            ni = int(ni_call[s, c])
            nb = int(nblk_call[s, c])
            if ni == 0:
                continue
            idx_all[:, icol : icol + ni // 16] = _wrap16(idx_pad[b, :ni])
            blk = rel_pad[b, : nb * P].reshape(nb, P).T  # [P, nb]
            rel_cols[:, col : col + nb] = blk.astype(np.float16)
            icol += ni // 16
            col += nb
    assert icol == icols and col == relcols

    ns, nd = pre["ns"], pre["nd"]
    nds12 = np.zeros((P, SLICES_CORE), dtype=np.float32)
    nd3 = np.zeros((P, SLICES_CORE), dtype=np.float32)
    mask = pre["core_of"] == core
    sl = pre["sloc_of"][mask]
    st = pre["slot_of"][mask]
    nds12[st, sl] = (nd * ns)[mask]
    nd3[st, sl] = nd[mask]
    return idx_all, rel_cols, nds12, nd3


def _build_program(pre):
    ni_call = pre["ni_call"]
    nblk_call = pre["nblk_call"]
    icols, relcols = _geometry(pre)

    nc = bacc.Bacc(
        "TRN2",
        target_bir_lowering=False,
        debug=False,
        num_devices=NCORES,
        num_swdge_queues=_NQUEUES,
    )
    dt = mybir.dt

    xn0_in = nc.declare_dram_parameter("xn0", [N_PAD, F_IN], FP16, isOutput=False)
    idx_in = nc.declare_dram_parameter("idx", [P, icols], dt.int16, isOutput=False)
    rel_in = nc.declare_dram_parameter("rel", [P, relcols], FP16, isOutput=False)
    iota_in = nc.declare_dram_parameter(
        "iotaw", [P, MAXT * NCHUNK * P], FP16, isOutput=False
    )
    w1_in = nc.declare_dram_parameter("w1", [F_IN, F_HID], FP16, isOutput=False)
    w2_in = nc.declare_dram_parameter("w2", [F_HID, F_HID], FP16, isOutput=False)
    w3_in = nc.declare_dram_parameter("w3", [F_HID, F_OUT], FP16, isOutput=False)
    b3_in = nc.declare_dram_parameter("b3rep", [P, F_OUT], dt.float32, isOutput=False)
    nds12_in = nc.declare_dram_parameter(
        "nds12", [P, SLICES_CORE], dt.float32, isOutput=False
    )
    nd3_in = nc.declare_dram_parameter(
        "nd3", [P, SLICES_CORE], dt.float32, isOutput=False
    )
    out_ext = nc.declare_dram_parameter("out", [PER_CORE, F_OUT], dt.float32, isOutput=True)

    with tile.TileContext(nc) as tc:
        with (
            tc.tile_pool(name="consts", bufs=1) as consts,
            tc.tile_pool(name="gt", bufs=int(_os.environ.get("GNN_GT_BUFS", "3"))) as gtp,
            tc.tile_pool(name="work", bufs=int(_os.environ.get("GNN_WORK_BUFS", "4"))) as work,
            tc.tile_pool(name="outw", bufs=3) as outw,
            tc.tile_pool(
                name="psum", bufs=int(_os.environ.get("GNN_PSUM_BUFS", "4")), space="PSUM"
            ) as psum,
            tc.tile_pool(name="psw", bufs=2, space="PSUM") as psw,
            tc.tile_pool(name="dram", bufs=1, space="DRAM") as dram,
        ):
            nc.gpsimd.load_library(library_config.mlp)

            idx_t = consts.tile([P, icols], dt.int16)
            rel_t = consts.tile([P, relcols], FP16)
            iota_t = consts.tile([P, MAXT * NCHUNK, P], FP16)
            w1_t = consts.tile([F_IN, F_HID], FP16)
            w2_t = consts.tile([F_HID, F_HID], FP16)
            w3_t = consts.tile([F_HID, F_OUT], FP16)
            b3_t = consts.tile([P, F_OUT], dt.float32)
            nds12_t = consts.tile([P, SLICES_CORE], dt.float32)
            nd3_t = consts.tile([P, SLICES_CORE], dt.float32)
            nc.sync.dma_start(out=idx_t[:], in_=idx_in[:])
            nc.sync.dma_start(out=rel_t[:], in_=rel_in[:])
            nc.sync.dma_start(
                out=iota_t[:], in_=iota_in[:].rearrange("p (t j) -> p t j", j=P)
            )
            nc.sync.dma_start(out=w1_t[:], in_=w1_in[:])
            nc.sync.dma_start(out=w2_t[:], in_=w2_in[:])
            nc.sync.dma_start(out=w3_t[:], in_=w3_in[:])
            nc.sync.dma_start(out=b3_t[:], in_=b3_in[:])
            nc.sync.dma_start(out=nds12_t[:], in_=nds12_in[:])
            nc.sync.dma_start(out=nd3_t[:], in_=nd3_in[:])

            xn0 = [
                xn0_in[c * CHUNK_ROWS : (c + 1) * CHUNK_ROWS, :] for c in range(NCHUNK)
            ]

            meta = (ni_call, nblk_call)
            args = (gtp, work, outw, psum, psw, idx_t, rel_t, iota_t, b3_t, out_ext)

            import contextlib

            loop_cm = (
                tc.For_i(0, _DBG_REPS, 1) if _DBG_REPS > 1 else contextlib.nullcontext()
            )
            with loop_cm:
                for r in range(_DBG_UNROLL):
                    ag_in1 = dram.tile(
                        [PER_CORE, F_HID], FP16, tag=f"ag_in1_r{r}", name=f"ag_in1_r{r}"
                    )
                    ag_in2 = dram.tile(
                        [PER_CORE, F_HID], FP16, tag=f"ag_in2_r{r}", name=f"ag_in2_r{r}"
                    )
                    xn1 = [
                        dram.tile(
                            [CHUNK_ROWS, F_HID],
                            FP16,
                            tag=f"xn1_{q}_r{r}",
                            name=f"xn1_{q}_r{r}",
                            addr_space="Shared",
                        )
                        for q in range(NSPLIT)
                    ]
                    xn2 = [
                        dram.tile(
                            [CHUNK_ROWS, F_HID],
                            FP16,
                            tag=f"xn2_{q}_r{r}",
                            name=f"xn2_{q}_r{r}",
                            addr_space="Shared",
                        )
                        for q in range(NSPLIT)
                    ]
                    layers = [
                        (xn0, w1_t, F_HID, nds12_t, ag_in1, xn1),
                        (xn1, w2_t, F_HID, nds12_t, ag_in2, xn2),
                        (xn2, w3_t, F_OUT, nd3_t, None, None),
                    ]
                    _emit_layers(nc, tc, layers, meta, args)
    nc.compile()
    return nc


def _emit_layers(nc, tc, layers, meta, args):
    dt = mybir.dt
    ni_call, nblk_call = meta
    gtp, work, outw, psum, psw, idx_t, rel_t, iota_t, b3_t, out_ext = args

    for li, (table, w_t, fo, scale_t, ag_in, ag_out) in enumerate(layers):
        if li >= _DBG_LAYERS:
            break
        with nc.named_scope(f"layer{li + 1}"):
            icol = 0
            col = 0
            ag_jobs = []
            for q in range(NSPLIT):
                for j in range(SLICES_SEG):
                    s = q * SLICES_SEG + j
                    gts = []
                    for c in range(NCHUNK):
                        ni = int(ni_call[s, c])
                        nb = int(nblk_call[s, c])
                        if ni == 0:
                            gts.append(None)
                            continue
                        gt = gtp.tile([P, MAXT, P], FP16, tag=f"gt{c}")
                        if _SKIP_GATHER:
                            nc.gpsimd.memset(gt[:, 0:1, 0:4], 0)
                        else:
                            nc.gpsimd.dma_gather(
                                gt[:, :nb, :],
                                table[c][:],
                                idx_t[:, icol : icol + ni // 16],
                                ni,
                                ni,
                                P,
                                queue_num=(c + NCHUNK * (s % (_NQUEUES // NCHUNK)))
                                if _NQUEUES > NCHUNK
                                else c,
                            )
                        icol += ni // 16
                        gts.append(gt)
                    if _SKIP_COMPUTE:
                        col += int(nblk_call[s].sum())
                        continue
                    nt = int(nblk_call[s].sum())
                    if _SKIP_ONEHOT:
                        sw = iota_t
                    else:
                        sw = work.tile([P, MAXT * NCHUNK, P], FP16, tag="sw")
                        oh_engine = (
                            nc.gpsimd
                            if (_OH_POOL_EVERY and s % _OH_POOL_EVERY == 1)
                            else nc.vector
                        )
                        oh_engine.tensor_tensor(
                            out=sw[:, :nt, :],
                            in0=iota_t[:, :nt, :],
                            in1=rel_t[:, col : col + nt].unsqueeze(2).broadcast_to(
                                (P, nt, P)
                            ),
                            op=mybir.AluOpType.is_equal,
                        )
                    acc = psum.tile([P, P], dt.float32, space="PSUM", tag="acc")
                    k = 0
                    for c in range(NCHUNK):
                        nb = int(nblk_call[s, c])
                        for b in range(nb):
                            nc.tensor.matmul(
                                out=acc[:],
                                lhsT=gts[c][:, b, :],
                                rhs=sw[:, k, :],
                                start=(k == 0),
                                stop=(k == nt - 1),
                            )
                            k += 1
                    col += nt
                    aggT = outw.tile([P, P], FP16, tag="aggT")
                    nc.scalar.activation(
                        out=aggT[:], in_=acc[:], func=mybir.ActivationFunctionType.Copy
                    )
                    op = psw.tile([P, fo], dt.float32, space="PSUM", tag="op")
                    nc.tensor.matmul(
                        out=op[:], lhsT=aggT[:], rhs=w_t[:], start=True, stop=True
                    )
                    if li < 2:
                        o = outw.tile([P, fo], FP16, tag="o")
                        nc.scalar.activation(
                            out=o[:],
                            in_=op[:],
                            func=mybir.ActivationFunctionType.Relu,
                            scale=scale_t[:, s : s + 1],
                        )
                        nc.sync.dma_start(out=ag_in[s * P : (s + 1) * P, :], in_=o[:])
                    else:
                        o = outw.tile([P, fo], dt.float32, tag="o")
                        nc.scalar.activation(
                            out=o[:],
                            in_=op[:],
                            func=mybir.ActivationFunctionType.Copy,
                            scale=scale_t[:, s : s + 1],
                        )
                        nc.vector.tensor_add(out=o[:], in0=o[:], in1=b3_t[:])
                        nc.sync.dma_start(
                            out=out_ext[s * P : (s + 1) * P, :], in_=o[:]
                        )
                if ag_in is not None and not _DBG_NO_AG and (
                    _DBG_REPS == 1 or _AG_IN_LOOP
                ):
                    ag_jobs.append(q)
            # Collectives are emitted at the END of the layer's Pool stream:
            # an AG's semaphore pre-wait (on its quarter's compute) must not
            # stall the issue of later quarters' gathers. Data deps are
            # unchanged: next-layer chunk-q gathers wait on AG_q completion.
            for q in ag_jobs:
                nc.gpsimd.collective_compute(
                    "AllGather",
                    mybir.AluOpType.bypass,
                    replica_groups=[list(range(NCORES))],
                    ins=[ag_in[q * SEG_ROWS : (q + 1) * SEG_ROWS, :].opt()],
                    outs=[ag_out[q][:].opt()],
                )


def _make_in_maps(x, src, dst, W1, W2, W3, b3):
    pre = _preprocess(src, dst)

    xn0 = np.zeros((N_PAD, F_IN), dtype=np.float16)
    xn0[pre["trow"]] = (x * pre["ns"][:, None]).astype(np.float16)
    iota = np.broadcast_to(
        np.arange(P, dtype=np.float16), (P, MAXT * NCHUNK, P)
    ).reshape(P, MAXT * NCHUNK * P).copy()
    b3rep = np.broadcast_to(b3, (P, F_OUT)).astype(np.float32).copy()

    in_maps = []
    for c in range(NCORES):
        idx_all, rel_cols, nds12, nd3 = _core_arrays(c, pre)
        in_maps.append(
            {
                "xn0": xn0,
                "idx": idx_all,
                "rel": rel_cols,
                "iotaw": iota,
                "w1": W1.astype(np.float16),
                "w2": W2.astype(np.float16),
                "w3": W3.astype(np.float16),
                "b3rep": b3rep,
                "nds12": nds12,
                "nd3": nd3,
            }
        )
    return in_maps, pre


def kernel(x, src, dst, W1, W2, W3, b3):
    x = np.ascontiguousarray(np.asarray(x, dtype=np.float32))
    src = np.asarray(src).astype(np.int64)
    dst = np.asarray(dst).astype(np.int64)

    in_maps, pre = _make_in_maps(x, src, dst, W1, W2, W3, b3)
    nc = _build_program(pre)
    res = run_bass_kernel_spmd(nc, in_maps, list(range(NCORES)))

    full = np.concatenate([res.results[c]["out"] for c in range(NCORES)], axis=0)
    return full[pre["catrow"]].astype(np.float32)
